# revision 16
# baseline (speedup 1.0000x reference)
"""Trainium2 Bass kernel for DiffusionTimePredictor.

Computes, for each (b,h) attention head:
  scores = Q @ K^T / sqrt(64)            (2048 x 2048)
  mean_sim = mean(scores)
  max_sim  = mean(rowmax(scores))
  entropy_proxy = mean(var_unbiased(softmax(scores), axis=-1))
  -> 3 features -> tiny MLP -> t = exp(clip(logit))

Sharding: 32 (b,h) heads data-parallel over 8 NeuronCores (4 heads/core).
MLP params replicated; each core computes its own 4 outputs on device.

Design (v2).  The scalar (ACT) engine is the hard floor: every score
element must be evacuated from PSUM exactly once, and ACT does it cheapest
(0.833ns/elem/partition).  One exp pass per [128,2048] tile (1892ns) with
the row-sum accumulator (+187ns) giving Z.  Everything else is kept
strictly under that 2079ns/tile budget on the other engines:

- PE: scores via two 64-contraction streams (tile_position trick), ~850ns.
- DVE per tile: SQSUM_PAIR_ACC custom op (e_lo^2+e_hi^2, sum accumulator)
  gives S2 = sum(exp(2s)) in one 1024-wide op (1127ns); a second custom op
  MAXPAIR_MAXACC (max(pair), MAX accumulator) on its output gives
  rowmax(e^2) in one 512-wide op (~600ns), replacing the old 4-op cascade.
- mean_sim: fp32-exact via column sums of the ORIGINAL fp32 Q/K (natural
  layout DMA + strided DVE reduce + ones-matmul in the tail), replacing the
  bf16 colsum path (was the largest feature error, 5e-2 -> ~1e-6).
- The tail uses ONLY the Exp activation table (one LoadActFuncSet for the
  whole kernel): rowmax ln via the int32-bitcast log trick on DVE,
  gelu via x*sigmoid(1.702x) (sigmoid from the Exp table + DVE reciprocal),
  final exp fused with the b3 bias add, clip applied in value space after
  the exp (monotone, so identical).
- Startup: head 0's cast/transpose is split in halves so the first score
  matmul starts ~9us earlier; MLP-param and colsum DMAs are queued after
  all transposes (they are only needed in the tail).

Engine notes baked in (measured on the cost model, verified on HW): ACT has
no fast modes (0.833ns/elem, +187ns accumulator read, +1283ns table load);
DVE runs 2-byte all-SBUF tensor_tensor at 2x but tensor_reduce and custom
(ISA) ops always at 1x; PE streams one moving column per 0.417ns per
64-contraction stream pair; tensor_tensor_reduce hangs inside For_i; the
Pool engine cannot run float tensor ops on this toolchain.
"""

import math

import numpy as np

import concourse.bacc as bacc
import concourse.bass as bass
import concourse.mybir as mybir
import concourse.tile as tile

F32 = mybir.dt.float32
BF16 = mybir.dt.bfloat16
I32 = mybir.dt.int32
AF = mybir.ActivationFunctionType
ALU = mybir.AluOpType
AX = mybir.AxisListType


def _register_dve_op(name, make_spec):
    """Register a custom DVE op via the documented dve_ops extension point."""
    import concourse.dve_ops as dve_ops
    from concourse.dve_spec import lower
    from concourse.dve_uop import DveOpSpec

    for op in dve_ops.OPS:
        if op.name == name:
            return op
    spec = make_spec()
    row = dve_ops._CUSTOM_DVE_ROW_BASE + len(dve_ops.OPS)
    shas = {}
    for ver in ("v3", "v4"):
        tmp = DveOpSpec(name=name, opcode=row, uops=lower(spec, ver=ver), rd1_en=True)
        shas[ver] = tmp.sha(ver)
    op = dve_ops.DveOp(name, spec, subdim=False, uops_sha=shas)
    dve_ops._SUB_OPCODE_FOR_NAME[name] = row
    dve_ops.OPS.append(op)
    dve_ops.CUSTOM_DVE_SPECS[name] = spec
    return op


def _sqsum_spec():
    """out[k] = in0[k]^2 + in1[k]^2, accum_out = sum(out).  Computes
    sum(e^2) over a score row in ONE 1024-wide DVE instruction; the squared
    output feeds the rowmax path at half width (max(e)^2 == max(e^2))."""
    from concourse.dve_spec import Spec, Src0, Src1, sq, AluOp

    return Spec(
        body=sq(Src0) + sq(Src1),
        accum=AluOp.ADD,
        reference=lambda in0, in1, s0, s1, imm2: (
            in0 * in0 + in1 * in1,
            (in0 * in0 + in1 * in1).sum(axis=-1, keepdims=True),
        ),
    )


def _maxpair_spec():
    """out[k] = max(in0[k], in1[k]), accum_out = max(out).  Full row max in
    ONE 512-wide DVE instruction on the SQSUM output pairs (replaces a
    3-level tensor_max cascade + reduce_max)."""
    from concourse.dve_spec import Spec, Src0, Src1, maxx, AluOp

    return Spec(
        body=maxx(Src0, Src1),
        accum=AluOp.MAX,
        reference=lambda in0, in1, s0, s1, imm2: (
            np.maximum(in0, in1),
            np.maximum(in0, in1).max(axis=-1, keepdims=True),
        ),
    )


SQSUM = _register_dve_op("SQSUM_PAIR_ACC", _sqsum_spec)
MAXPAIR = _register_dve_op("MAXPAIR_MAXACC", _maxpair_spec)

B, H, SQ, SK, D = 4, 8, 2048, 2048, 64
NCORES = 8
BH = B * H
BHC = BH // NCORES  # heads per core = 4
NQT = SQ // 128  # q tiles per head = 16
LOG_T_MIN = math.log(0.1)
LOG_T_MAX = math.log(10.0)

# int32-bitcast ln trick: ln(x) ~ (bits_f32(x) - LOG_B) * LOG_C, mid-corrected
LOG_C = math.log(2.0) / (1 << 23)  # 8.2629582e-08
LOG_B = 1064866805.0

_CACHE = {}
LAST_RESULTS = None


def _build(repeat=None):
    """Build the per-core bass module.  With repeat=K, the whole computation
    is wrapped in a Tile For_i loop executing K times — used only for device
    timing (marginal time per iteration removes host dispatch overhead)."""
    from contextlib import nullcontext

    nc = bacc.Bacc("TRN2", target_bir_lowering=False, debug=False)

    q_d = nc.dram_tensor("q", [BHC, SQ, D], F32, kind="ExternalInput")
    k_d = nc.dram_tensor("k", [BHC, SK, D], F32, kind="ExternalInput")
    w1_d = nc.dram_tensor("W1", [3, 64], F32, kind="ExternalInput")
    b1_d = nc.dram_tensor("b1", [64], F32, kind="ExternalInput")
    w2_d = nc.dram_tensor("W2", [64, 64], F32, kind="ExternalInput")
    b2_d = nc.dram_tensor("b2", [64], F32, kind="ExternalInput")
    w3_d = nc.dram_tensor("W3", [64, 1], F32, kind="ExternalInput")
    b3_d = nc.dram_tensor("b3", [1], F32, kind="ExternalInput")
    out_d = nc.dram_tensor("out", [BHC], F32, kind="ExternalOutput")
    # features as [mean_sim(4) | max_sim(4) | entropy(4)] flat
    featdbg_d = nc.dram_tensor("featdbg", [3, BHC], F32, kind="ExternalOutput")

    # bf16 cast scratch: separate DRAM tensors per head so the tile
    # framework never invents write-after-read deps between one head's
    # cast and another's transpose.
    scq_p = [nc.dram_tensor(f"scq{bh}", [SQ, D], BF16) for bh in range(BHC)]
    sck_p = [nc.dram_tensor(f"sck{bh}", [SK, D], BF16) for bh in range(BHC)]

    # persistent SBUF state
    zbias = nc.alloc_sbuf_tensor("zbias", [128, 1], F32).ap()
    onesf = nc.alloc_sbuf_tensor("onesf", [128, 1], F32).ap()
    Zt = nc.alloc_sbuf_tensor("Zt", [128, BHC * NQT], F32).ap()    # row sum e
    S2s = nc.alloc_sbuf_tensor("S2s", [128, BHC * NQT], F32).ap()  # row sum e^2
    Mt = nc.alloc_sbuf_tensor("Mt", [128, BHC * NQT], F32).ap()    # rowmax e^2
    # staging scratch for the reduce passes (DVE is serial, so single
    # buffers are race-free)
    sq1 = nc.alloc_sbuf_tensor("sq1", [128, 1024], BF16).ap()   # e^2 pairs
    m2 = nc.alloc_sbuf_tensor("m2", [128, 512], BF16).ap()      # maxpair out
    fin = nc.alloc_sbuf_tensor("fin", [128, 32 * BHC], F32).ap()
    rcp = nc.alloc_sbuf_tensor("rcp", [128, NQT], F32).ap()
    t1s = nc.alloc_sbuf_tensor("t1s", [128, NQT], F32).ap()
    # fp32 colsum partials: per head [q(64) | k(64)]
    qkp = nc.alloc_sbuf_tensor("qkp", [128, 128 * BHC], F32).ap()
    Fs = nc.alloc_sbuf_tensor("Fs", [1, 32 * BHC], F32).ap()
    qks = nc.alloc_sbuf_tensor("qks", [1, 128 * BHC], F32).ap()
    Ss = nc.alloc_sbuf_tensor("Ss", [1, 2 * BHC], F32).ap()
    dt4 = nc.alloc_sbuf_tensor("dt4", [1, 64 * BHC], F32).ap()
    g2 = nc.alloc_sbuf_tensor("g2", [1, BHC], F32).ap()
    stg = nc.alloc_sbuf_tensor("stg", [1, 3 * BHC], F32).ap()
    w1r = nc.alloc_sbuf_tensor("w1r", [1, 3 * 64], F32).ap()
    b1s = nc.alloc_sbuf_tensor("b1s", [64, 1], F32).ap()
    nb1 = nc.alloc_sbuf_tensor("nb1", [64, 1], F32).ap()
    w2s = nc.alloc_sbuf_tensor("w2s", [64, 64], F32).ap()
    b2s = nc.alloc_sbuf_tensor("b2s", [64, 1], F32).ap()
    nb2 = nc.alloc_sbuf_tensor("nb2", [64, 1], F32).ap()
    w3s = nc.alloc_sbuf_tensor("w3s", [64, 1], F32).ap()
    b3s = nc.alloc_sbuf_tensor("b3s", [1, 1], F32).ap()
    h1x = nc.alloc_sbuf_tensor("h1x", [64, BHC], F32).ap()
    h1e = nc.alloc_sbuf_tensor("h1e", [64, BHC], F32).ap()
    h1r = nc.alloc_sbuf_tensor("h1r", [64, BHC], F32).ap()
    h1i = nc.alloc_sbuf_tensor("h1i", [64, BHC], F32).ap()
    h1v = nc.alloc_sbuf_tensor("h1v", [64, BHC], F32).ap()
    h2x = nc.alloc_sbuf_tensor("h2x", [64, BHC], F32).ap()
    h2e = nc.alloc_sbuf_tensor("h2e", [64, BHC], F32).ap()
    h2r = nc.alloc_sbuf_tensor("h2r", [64, BHC], F32).ap()
    h2i = nc.alloc_sbuf_tensor("h2i", [64, BHC], F32).ap()
    h2v = nc.alloc_sbuf_tensor("h2v", [64, BHC], F32).ap()
    texp = nc.alloc_sbuf_tensor("texp", [1, BHC], F32).ap()
    tclip = nc.alloc_sbuf_tensor("tclip", [1, BHC], F32).ap()

    with tile.TileContext(nc) as tc:
      with tc.For_i(0, repeat, 1) if repeat else nullcontext():
        with (
            tc.tile_pool(name="tr", bufs=3) as trp,
            tc.tile_pool(name="ep", bufs=8) as epp,
            tc.tile_pool(name="qn", bufs=2) as qnp,
            tc.tile_pool(name="psB", bufs=2, space=bass.MemorySpace.PSUM) as psB,
        ):
            nc.vector.memset(zbias, 0.0)
            nc.vector.memset(onesf, 1.0)

            # --- casts + transposes, interleaved per head so head h's
            # transposes outrank head h+1's casts in scheduler priority
            # (the DMA pipe is globally serialized; order is everything).
            QTs_all, KTs_all, QTw_all = [], [], []
            for bh in range(BHC):
                nc.gpsimd.dma_start(out=sck_p[bh][:], in_=k_d[bh][:])
                nc.gpsimd.dma_start(out=scq_p[bh][:], in_=q_d[bh][:])
                QT = trp.tile([128, SQ // 2], BF16, tag="QT")
                KT = trp.tile([128, SK // 2], BF16, tag="KT")
                QTw = trp.tile([128, SQ // 2], BF16, tag="QTw")
                QTs_all.append(QT); KTs_all.append(KT); QTw_all.append(QTw)
                nc.sync.dma_start_transpose(
                    KT[:], sck_p[bh][:].rearrange("(a b) d -> a (b d)", b=2)
                )
                nc.sync.dma_start_transpose(
                    QT[:], scq_p[bh][:].rearrange("(a b) d -> a (b d)", b=2)
                )
                nc.sync.dma_start(out=QTw[64:128, :], in_=QT[0:64, :])
                nc.sync.dma_start(out=QTw[0:64, :], in_=QT[64:128, :])

            # --- fp32 colsums of the ORIGINAL Q/K for mean_sim, and MLP
            # params.  Tail-only: queued on the gpsimd queue behind all the
            # casts, and held back with a scheduler wait so they never steal
            # DMA-engine bandwidth from the critical transpose window.
            with tc.tile_wait_until(0.055):
                for bh in range(BHC):
                    for j, src in ((0, q_d), (1, k_d)):
                        qn = qnp.tile([128, SQ * D // 128], F32, tag="qn")
                        nc.gpsimd.dma_start(out=qn[:], in_=src[bh][:])
                        # partition p holds rows 16p..16p+15; strided view
                        # puts rows innermost so reduce_sum sums rows per d.
                        nc.vector.reduce_sum(
                            qkp[:, 128 * bh + 64 * j: 128 * bh + 64 * j + 64],
                            qn[:].rearrange("p (r d) -> p d r", d=64),
                            axis=AX.X,
                        )
                nc.gpsimd.dma_start(
                    out=w1r, in_=w1_d[:].rearrange("f m -> (f m)").rearrange(
                        "(a b) -> a b", a=1)
                )
                nc.gpsimd.dma_start(
                    out=b1s, in_=b1_d[:].rearrange("(a b) -> a b", b=1))
                nc.gpsimd.dma_start(out=w2s, in_=w2_d[:])
                nc.gpsimd.dma_start(
                    out=b2s, in_=b2_d[:].rearrange("(a b) -> a b", b=1))
                nc.gpsimd.dma_start(out=w3s, in_=w3_d[:])
                nc.gpsimd.dma_start(
                    out=b3s, in_=b3_d[:].rearrange("(a b) -> a b", b=1))
                nc.vector.tensor_scalar_mul(nb1, b1s, -1.702)
                nc.vector.tensor_scalar_mul(nb2, b2s, -1.702)

            # --- main loop: 16 q-tiles per head, ACT-bound
            for bh in range(BHC):
                QT, KT, QTw = QTs_all[bh], KTs_all[bh], QTw_all[bh]
                for t in range(NQT):
                    even = t < NQT // 2
                    ct = 128 * (t % (NQT // 2))
                    lo_stat = (QT if even else QTw)[0:64, ct: ct + 128]
                    hi_stat = (QTw if even else QT)[64:128, ct: ct + 128]
                    gt = bh * NQT + t

                    P = psB.tile([128, 2048], F32, tag="P")
                    nc.tensor.matmul(
                        P[:, 0:512], lo_stat, KT[0:64, 0:512], start=True, stop=True
                    )
                    nc.tensor.matmul(
                        P[:, 512:1024], lo_stat, KT[0:64, 512:1024],
                        start=True, stop=True,
                    )
                    nc.tensor.matmul(
                        P[:, 1024:1536], hi_stat, KT[64:128, 0:512],
                        start=True, stop=True, tile_position=(64, 0),
                    )
                    nc.tensor.matmul(
                        P[:, 1536:2048], hi_stat, KT[64:128, 512:1024],
                        start=True, stop=True, tile_position=(64, 0),
                    )

                    e = epp.tile([128, 2048], BF16, tag="e")
                    nc.scalar.activation(
                        e[:], P[:], AF.Exp, bias=zbias, scale=0.125,
                        accum_out=Zt[:, gt: gt + 1],
                    )
                    # S2 (sum e^2) + squared pairs in one 1024-wide custom op
                    nc.vector._custom_dve(
                        SQSUM, out=sq1[:],
                        in0=e[:, 0:1024], in1=e[:, 1024:2048],
                        accum_out=S2s[:, gt: gt + 1],
                    )
                    # rowmax(e^2) in one 512-wide custom op (MAX accumulator)
                    nc.vector._custom_dve(
                        MAXPAIR, out=m2[:],
                        in0=sq1[:, 0:512], in1=sq1[:, 512:1024],
                        accum_out=Mt[:, gt: gt + 1],
                    )
                    if t == NQT - 1:
                        # head stats complete: sp2 = S2/Z^2 prep and the
                        # ln-bit-trick rowmax both overlap the next head.
                        cb = bh * NQT
                        nc.vector.reciprocal(rcp, Zt[:, cb: cb + NQT])
                        nc.vector.tensor_mul(t1s, S2s[:, cb: cb + NQT], rcp)
                        nc.vector.tensor_mul(
                            fin[:, bh * 32: bh * 32 + 16], t1s, rcp
                        )
                        # rowmax(s) = 0.5*ln(rowmax e^2) via int32 bitcast
                        nc.vector.tensor_scalar(
                            fin[:, bh * 32 + 16: bh * 32 + 32],
                            Mt[:, cb: cb + NQT].bitcast(I32),
                            scalar1=0.5 * LOG_C, scalar2=-0.5 * LOG_C * LOG_B,
                            op0=ALU.mult, op1=ALU.add,
                        )

        # ---------------- tail: features + MLP (Exp table only) -----------
        with tc.tile_pool(name="psT", bufs=1, space=bass.MemorySpace.PSUM) as psT:
            # mean_sim: colsum dot via ones-matmul + paired mul/reduce
            PCS = psT.tile([1, 128 * BHC], F32, tag="PCS")
            nc.tensor.matmul(PCS[:], onesf, qkp, start=True, stop=True)
            nc.vector.tensor_copy(qks, PCS[:])
            for bh in range(BHC):
                nc.vector.tensor_mul(
                    dt4[:, 64 * bh: 64 * bh + 64],
                    qks[0:1, 128 * bh: 128 * bh + 64],
                    qks[0:1, 128 * bh + 64: 128 * bh + 128],
                )
            nc.vector.reduce_sum(
                g2, dt4.rearrange("p (h d) -> p h d", d=64), axis=AX.X
            )

            PF = psT.tile([1, 32 * BHC], F32, tag="PF")
            nc.tensor.matmul(PF[:], onesf, fin, start=True, stop=True)
            nc.vector.tensor_copy(Fs, PF[:])
            nc.vector.reduce_sum(
                Ss, Fs.rearrange("p (g c) -> p g c", c=16), axis=AX.X
            )
            # features:
            # mean_sim = dot/(8*SQ*SK)
            nc.vector.tensor_scalar_mul(
                stg[:, 0:BHC], g2, 1.0 / (8.0 * SQ * SK)
            )
            # max_sim = mean over rows of rowmax(s)  (fin already holds ln)
            nc.vector.tensor_scalar_mul(
                stg[:, BHC: 2 * BHC], Ss[:, 1: 2 * BHC: 2], 1.0 / SQ
            )
            # entropy = (sum(sp2) - 1) / (SK*(SK-1))
            cent = 1.0 / (float(SK) * (SK - 1.0))
            nc.vector.tensor_scalar(
                stg[:, 2 * BHC: 3 * BHC], Ss[:, 0: 2 * BHC: 2],
                scalar1=cent, scalar2=-cent, op0=ALU.mult, op1=ALU.add,
            )
            nc.sync.dma_start(out=featdbg_d[:], in_=stg)

            # MLP layer 1: PM1 = sum_f W1[f,:] (x) feat_f, gelu via sigmoid
            PM1 = psT.tile([64, BHC], F32, tag="PM1")
            for f in range(3):
                nc.tensor.matmul(
                    PM1[:], w1r[0:1, 64 * f: 64 * f + 64],
                    stg[:, f * BHC: (f + 1) * BHC],
                    start=(f == 0), stop=(f == 2),
                )
            # gelu(x+b) = (x+b) * sigmoid(1.702(x+b)) with sigmoid from the
            # Exp table: h = exp(-1.702(x+b)); gelu = (x+b) / (1+h)
            nc.scalar.activation(h1e, PM1[:], AF.Exp, bias=nb1, scale=-1.702)
            nc.vector.tensor_scalar(h1r, h1e, scalar1=1.0, scalar2=None,
                                    op0=ALU.add)
            nc.vector.reciprocal(h1i, h1r)
            nc.vector.scalar_tensor_tensor(
                out=h1v, in0=PM1[:], scalar=b1s, in1=h1i,
                op0=ALU.add, op1=ALU.mult,
            )

            PM2 = psT.tile([64, BHC], F32, tag="PM2")
            nc.tensor.matmul(PM2[:], w2s, h1v, start=True, stop=True)
            nc.scalar.activation(h2e, PM2[:], AF.Exp, bias=nb2, scale=-1.702)
            nc.vector.tensor_scalar(h2r, h2e, scalar1=1.0, scalar2=None,
                                    op0=ALU.add)
            nc.vector.reciprocal(h2i, h2r)
            nc.vector.scalar_tensor_tensor(
                out=h2v, in0=PM2[:], scalar=b2s, in1=h2i,
                op0=ALU.add, op1=ALU.mult,
            )

            PM3 = psT.tile([1, BHC], F32, tag="PM3")
            nc.tensor.matmul(PM3[:], w3s, h2v, start=True, stop=True)
            # t = exp(logit + b3), clipped in value space (exp is monotone)
            nc.scalar.activation(texp, PM3[:], AF.Exp, bias=b3s, scale=1.0)
            nc.vector.tensor_scalar(
                tclip, texp, scalar1=math.exp(LOG_T_MIN),
                scalar2=math.exp(LOG_T_MAX), op0=ALU.max, op1=ALU.min,
            )
            nc.sync.dma_start(
                out=out_d[:].rearrange("(a b) -> a b", a=1), in_=tclip
            )

    nc.compile()
    return nc


class _Runner:
    """Caches the jitted shard_map executable for the compiled bass module so
    repeated invocations (timing loops) don't re-trace/re-compile.  Mirrors
    concourse.bass2jax.run_bass_via_pjrt's multi-core path."""

    def __init__(self, nc, n_cores):
        import jax
        from jax.sharding import Mesh, PartitionSpec
        from jax.experimental.shard_map import shard_map
        from concourse import bass2jax as b2j

        b2j.install_neuronx_cc_hook()
        self.nc = nc
        self.n_cores = n_cores
        in_names, out_names, out_avals, zero_outs = [], [], [], []
        partition_name = (
            nc.partition_id_tensor.name if nc.partition_id_tensor else None
        )
        for alloc in nc.m.functions[0].allocations:
            if not isinstance(alloc, mybir.MemoryLocationSet):
                continue
            name = alloc.memorylocations[0].name
            if alloc.kind == "ExternalInput":
                if name != partition_name:
                    in_names.append(name)
            elif alloc.kind == "ExternalOutput":
                out_names.append(name)
                shape = tuple(alloc.tensor_shape)
                dtype = mybir.dt.np(alloc.dtype)
                out_avals.append(jax.core.ShapedArray(shape, dtype))
                zero_outs.append(np.zeros(shape, dtype))
        n_params = len(in_names)
        n_outs = len(out_avals)
        in_names = in_names + out_names
        if partition_name is not None:
            in_names.append(partition_name)
        self.in_names = in_names
        self.out_names = out_names
        self.out_avals = out_avals
        self.n_params = n_params
        self.zero_outs = zero_outs
        donate = tuple(range(n_params, n_params + n_outs))

        def _body(*args):
            operands = list(args)
            if partition_name is not None:
                operands.append(b2j.partition_id_tensor())
            outs = b2j._bass_exec_p.bind(
                *operands,
                out_avals=tuple(out_avals),
                in_names=tuple(in_names),
                out_names=tuple(out_names),
                lowering_input_output_aliases=(),
                sim_require_finite=True,
                sim_require_nnan=True,
                nc=nc,
            )
            return tuple(outs)

        devices = jax.devices()[:n_cores]
        self.mesh = Mesh(np.asarray(devices), ("core",))
        in_specs = (PartitionSpec("core"),) * (n_params + n_outs)
        out_specs = (PartitionSpec("core"),) * n_outs
        self._fn = jax.jit(
            shard_map(
                _body,
                mesh=self.mesh,
                in_specs=in_specs,
                out_specs=out_specs,
                check_rep=False,
            ),
            donate_argnums=donate,
            keep_unused=True,
        )
        self._jax = jax

    def concat_inputs(self, in_maps):
        per_core = [
            [np.asarray(m[name]) for name in self.in_names[: self.n_params]]
            for m in in_maps
        ]
        return [
            np.concatenate([per_core[c][i] for c in range(self.n_cores)], axis=0)
            for i in range(self.n_params)
        ]

    def _zeros(self):
        return [
            np.zeros((self.n_cores * z.shape[0], *z.shape[1:]), z.dtype)
            for z in self.zero_outs
        ]

    def run(self, concat_in):
        out_arrs = self._fn(*concat_in, *self._zeros())
        out_arrs = [np.asarray(o) for o in out_arrs]
        return [
            {
                name: out_arrs[i].reshape(self.n_cores, *self.out_avals[i].shape)[c]
                for i, name in enumerate(self.out_names)
            }
            for c in range(self.n_cores)
        ]

    def time(self, concat_in, iters=30):
        import time as _time

        dev_in = [self._jax.device_put(x) for x in concat_in]
        # warmup (also triggers compile)
        self._fn(*dev_in, *self._zeros())[0].block_until_ready()
        times = []
        for _ in range(iters):
            zs = self._zeros()
            t0 = _time.perf_counter()
            out = self._fn(*dev_in, *zs)
            out[0].block_until_ready()
            times.append(_time.perf_counter() - t0)
        return times


def kernel(**inputs):
    global LAST_RESULTS
    if "nc" not in _CACHE:
        _CACHE["nc"] = _build()
        _CACHE["runner"] = _Runner(_CACHE["nc"], NCORES)
    nc = _CACHE["nc"]

    q = np.ascontiguousarray(np.asarray(inputs["query"], dtype=np.float32)).reshape(
        BH, SQ, D
    )
    k = np.ascontiguousarray(np.asarray(inputs["key"], dtype=np.float32)).reshape(
        BH, SK, D
    )
    shared = {
        "W1": np.ascontiguousarray(np.asarray(inputs["W1"], dtype=np.float32)),
        "b1": np.ascontiguousarray(np.asarray(inputs["b1"], dtype=np.float32)),
        "W2": np.ascontiguousarray(np.asarray(inputs["W2"], dtype=np.float32)),
        "b2": np.ascontiguousarray(np.asarray(inputs["b2"], dtype=np.float32)),
        "W3": np.ascontiguousarray(np.asarray(inputs["W3"], dtype=np.float32)),
        "b3": np.ascontiguousarray(np.asarray(inputs["b3"], dtype=np.float32)),
    }
    in_maps = []
    for c in range(NCORES):
        m = dict(shared)
        m["q"] = np.ascontiguousarray(q[c * BHC: (c + 1) * BHC])
        m["k"] = np.ascontiguousarray(k[c * BHC: (c + 1) * BHC])
        in_maps.append(m)

    runner = _CACHE["runner"]
    concat_in = runner.concat_inputs(in_maps)
    results = runner.run(concat_in)
    LAST_RESULTS = results
    t = np.concatenate([results[i]["out"] for i in range(NCORES)])
    return t.reshape(B, H, 1, 1).astype(np.float32)


def _make_in_maps(inputs):
    q = np.asarray(inputs["query"], dtype=np.float32).reshape(BH, SQ, D)
    k = np.asarray(inputs["key"], dtype=np.float32).reshape(BH, SK, D)
    shared = {
        n: np.ascontiguousarray(np.asarray(inputs[n], dtype=np.float32))
        for n in ("W1", "b1", "W2", "b2", "W3", "b3")
    }
    in_maps = []
    for c in range(NCORES):
        m = dict(shared)
        m["q"] = np.ascontiguousarray(q[c * BHC: (c + 1) * BHC])
        m["k"] = np.ascontiguousarray(k[c * BHC: (c + 1) * BHC])
        in_maps.append(m)
    return in_maps


def time_kernel(iters=30, **inputs):
    """Returns list of per-call wall times (s) for the cached executable."""
    kernel(**inputs)  # ensure built + correct path warm
    runner = _CACHE["runner"]
    return runner.time(runner.concat_inputs(_make_in_maps(inputs)), iters=iters)


def time_kernel_device(k_small=1, k_big=33, pipeline=24, reps=3, **inputs):
    """True device time per kernel execution.

    Builds two For_i-wrapped modules that run the whole computation K times
    on device; the marginal wall time per extra iteration, measured with a
    pipelined stream of dispatches, is the device execution time (host/axon
    dispatch overhead and I/O transfer cancel in the difference)."""
    import time as _time

    for key, K in (("rs", k_small), ("rb", k_big)):
        if key not in _CACHE:
            _CACHE[key] = _Runner(_build(repeat=K), NCORES)
    rs, rb = _CACHE["rs"], _CACHE["rb"]
    ci = rs.concat_inputs(_make_in_maps(inputs))
    dev_in = [rs._jax.device_put(x) for x in ci]

    def run_stream(r, n):
        outs = [r._fn(*dev_in, *r._zeros()) for _ in range(n)]
        outs[-1][0].block_until_ready()

    run_stream(rs, 2)
    run_stream(rb, 2)  # warm/compile
    t_s, t_b = [], []
    for _ in range(reps):
        t0 = _time.perf_counter()
        run_stream(rs, pipeline)
        t_s.append((_time.perf_counter() - t0) / pipeline)
        t0 = _time.perf_counter()
        run_stream(rb, pipeline)
        t_b.append((_time.perf_counter() - t0) / pipeline)
    ts_m = sorted(t_s)[len(t_s) // 2]
    tb_m = sorted(t_b)[len(t_b) // 2]
    per_exec = (tb_m - ts_m) / (k_big - k_small)
    return per_exec, ts_m, tb_m


def predict_timeline(trace_path=None):
    """Cost-model predicted kernel time in ns (single core), optional perfetto."""
    from concourse.timeline_sim import TimelineSim

    if "nc" not in _CACHE:
        _CACHE["nc"] = _build()
    ts = TimelineSim(_CACHE["nc"], trace=trace_path is not None)
    total = ts.simulate()
    if trace_path is not None and ts.perfetto is not None:
        ts.perfetto.save(trace_path)
    return total


if __name__ == "__main__":
    rng = np.random.default_rng(0)
    ins = {
        "query": rng.standard_normal((B, H, SQ, D), dtype=np.float32),
        "key": rng.standard_normal((B, H, SQ, D), dtype=np.float32),
        "W1": rng.standard_normal((3, 64), dtype=np.float32) * 0.1,
        "b1": np.zeros(64, np.float32),
        "W2": rng.standard_normal((64, 64), dtype=np.float32) * 0.1,
        "b2": np.zeros(64, np.float32),
        "W3": np.zeros((64, 1), np.float32),
        "b3": np.zeros(1, np.float32),
    }
    print(kernel(**ins))
